# revision 19
# baseline (speedup 1.0000x reference)
"""8-way sharded MultiHeadAttention for Trainium2 (Bass/Tile).

Problem: B=2, S=2048, H=1024, NH=16 heads of D=64.
  out = softmax((x@wq.T+bq) @ (x@wk.T+bk).T / sqrt(D)) @ (x@wv.T+bv),
  concat heads, @ wo.T + bo.

Sharding (Megatron-style tensor parallel over 8 NeuronCores):
  core c owns batch b = c//4 and the 4 heads 4*(c%4)..4*(c%4)+3
  (feature columns Ic = 256*(c%4) .. +256 of q/k/v).
  - column-parallel QKV projections, attention fully local per head,
  - row-parallel output projection producing a partial [H, S] result;
    the 4 partials per batch are summed on the host.

v3 design notes:
  - All matmul operands are fp16 (same 1 cyc/row PE rate as fp32r, half
    the DMA/SBUF footprint, 2x faster weight loads via FWL); PSUM
    accumulation stays fp32.
  - Attention runs in 512-query phases (2 head-pairs x 4 query chunks,
    16 key-chunks each).  Every attention PSUM tile is then ONE 2KB
    bank: the score buffer quadruple-buffers in 4 banks, the paired
    ctx accumulators take 2, and the remaining 2 banks form a
    DEDICATED background pool -- so the v projection, the m=1 (heads
    2/3) q/k projections, the out-projection and the 1/Z broadcasts
    never block the score/exp pipeline (in v2 they shared one ring and
    each background chunk stalled the exp stream).
  - x streams in k-chunks over 3 DGE queues; the m=0 (heads 0/1) q/k
    projections contract each chunk as it lands into 4 concurrent
    accumulators in a scoped 8-bank pool, so the first scores + exp
    issue right after the last chunk.  A dummy exp at t=0 preloads the
    ACT exp table set (~2.7us).
  - scores are computed transposed ([key, query]); the softmax sum
    folds into attn@v via a ones-augmented V ([v | 1]).  exp runs on
    the scalar engine straight out of PSUM with the 1/sqrt(D) scale
    fused.  No max-subtraction (|scores/8| < ~5.5, exp safe in fp32).
  - the head pair sits at prow 0/64, so the two K=64 score matmuls
    land in disjoint PE row groups (tile_position) and overlap on HW.
  - softmax normalization: 1/Z (DVE reciprocal, f32r) is broadcast
    across partitions by a K=1 ones matmul through the background pool
    and fused into the PSUM eviction multiply.  Each phase's first
    key-chunk is emitted before the previous phase's normalize so the
    exp stream never waits on it.
  - out-projection chunks are gated per query-chunk on the second
    head-pair's normalize and run as deprioritized background; only
    the last query chunk's 8 chunks trail the final exp (~6us tail).
"""

import sys

for _p in ("/opt/trn_rl_repo", "/root/.axon_site/_ro/trn_rl_repo"):
    if _p not in sys.path:
        sys.path.append(_p)

from contextlib import ExitStack

import numpy as np

import concourse.bass as bass
import concourse.mybir as mybir
import concourse.tile as tile
from concourse import bacc
from concourse.bass_utils import run_bass_kernel_spmd

F32 = mybir.dt.float32
F16 = mybir.dt.float16
F32R = mybir.dt.float32r
AF = mybir.ActivationFunctionType
ADD = mybir.AluOpType.add
MULT = mybir.AluOpType.mult

P = 128
B = 2
S = 2048          # tokens
H = 1024          # hidden
KO = H // P       # 8 k-chunks for the QKV projections
MO = 2            # 256 local features / 128
HEADS = 4         # heads per core
D = 64
NKT = S // P      # 16 key chunks
HALF = 1024
QC = 512          # query chunk width (one PSUM bank of fp32)
NQC = S // QC     # 4 query chunks
NCORES = 8
VPAD = 72         # per-(kt, head) v row stride in elems (65 used, 16B-aligned)
VLEAD = 2         # v-projection chunks emitted ahead of the attn@v consumer

# test.py can flip these before calling kernel()
TRACE = False
DEBUG = False
LAST_RESULT = {}


def build_mha_kernel(nc: bass.Bass):
    xT = nc.declare_dram_parameter("xT", [H, S], F16, isOutput=False)
    wqT = nc.declare_dram_parameter("wqT", [H, 256], F16, isOutput=False)
    wkT = nc.declare_dram_parameter("wkT", [H, 256], F16, isOutput=False)
    wvT = nc.declare_dram_parameter("wvT", [H, 256], F16, isOutput=False)
    bq2 = nc.declare_dram_parameter("bq2", [P, MO], F32, isOutput=False)
    bk2 = nc.declare_dram_parameter("bk2", [P, MO], F32, isOutput=False)
    bv2 = nc.declare_dram_parameter("bv2", [P, 256], F32, isOutput=False)
    woT = nc.declare_dram_parameter("woT", [256, H], F16, isOutput=False)
    vones_d = nc.declare_dram_parameter("vones_d", [P, NKT, HEADS, 1], F16,
                                        isOutput=False)
    ones_d = nc.declare_dram_parameter("ones_d", [P, 64], F32R,
                                       isOutput=False)
    poutT = nc.declare_dram_parameter("poutT", [H, S], F16, isOutput=True)
    if DEBUG:
        dbg_q = nc.declare_dram_parameter("dbg_q", [P, MO, S], F16,
                                          isOutput=True)
        dbg_k = nc.declare_dram_parameter("dbg_k", [P, MO, S], F16,
                                          isOutput=True)
        dbg_v = nc.declare_dram_parameter("dbg_v", [P, NKT, HEADS, VPAD], F16,
                                          isOutput=True)
        dbg_c = nc.declare_dram_parameter("dbg_c", [P, MO, S], F16,
                                          isOutput=True)

    xT_r = xT.rearrange("(o p) n -> p o n", p=P)        # [128, 8, 2048]
    wq_r = wqT.rearrange("(o p) m -> p o m", p=P)       # [128, 8, 256]
    wk_r = wkT.rearrange("(o p) m -> p o m", p=P)
    wv_r = wvT.rearrange("(o p) m -> p o m", p=P)
    wo_r = woT.rearrange("(o p) m -> p o m", p=P)       # [128, 2, 1024]
    pout_r = poutT.rearrange("(o p) n -> p o n", p=P)   # [128, 8, 2048]

    with tile.TileContext(nc) as tc, ExitStack() as ctx:
        xp = ctx.enter_context(tc.tile_pool(name="xp", bufs=1))
        wp = ctx.enter_context(tc.tile_pool(name="wp", bufs=1))
        qk = ctx.enter_context(tc.tile_pool(name="qk", bufs=1))
        vp = ctx.enter_context(tc.tile_pool(name="vp", bufs=1))
        cx = ctx.enter_context(tc.tile_pool(name="cx", bufs=1))
        pp = ctx.enter_context(tc.tile_pool(name="pp", bufs=8))
        nm = ctx.enter_context(tc.tile_pool(name="nm", bufs=3))
        ob = ctx.enter_context(tc.tile_pool(name="ob", bufs=6))

        x_sb = xp.tile([P, KO, S], F16)
        wq_sb = wp.tile([P, KO, 256], F16, tag="wq")
        wk_sb = wp.tile([P, KO, 256], F16, tag="wk")
        wv_sb = wp.tile([P, KO, 256], F16, tag="wv")
        wo_sb = wp.tile([P, MO, H], F16, tag="wo")
        bq_sb = wp.tile([P, MO], F32, tag="bq")
        bk_sb = wp.tile([P, MO], F32, tag="bk")
        bv_sb = wp.tile([P, 256], F32, tag="bv")

        qT_sb = qk.tile([P, MO, S], F16, tag="q")       # [feat, token]
        kT_sb = qk.tile([P, MO, S], F16, tag="k")
        v_sb = vp.tile([P, NKT, HEADS, VPAD], F16)      # [tok, kt, h, v|1|pad]
        ctx_sb = cx.tile([P, MO, S], F16)

        # ---- ACT exp table preload: one dummy exp at t=0 hides the
        # ~2.7us table-set load before the first real scores arrive
        dum = nm.tile([P, 8], F32, tag="dummy")
        nc.vector.memset(dum[0:1, :], 0.0)
        nc.scalar.activation(dum[0:1, :], dum[0:1, :], AF.Exp, scale=1.0)

        # ---- input DMA over 3 DGE queues: x chunks + the m=0 q/k weight
        # halves (they gate the first exp) + wv (gates the v trickle)
        # first; m=1 halves, wo and the ones columns behind them.
        qs = [nc.sync, nc.gpsimd, nc.scalar]
        for k in range(KO):
            qs[k % 3].dma_start(x_sb[:, k, :], xT_r[:, k, :])
            qs[(k + 1) % 3].dma_start(wq_sb[:, k, 0:P], wq_r[:, k, 0:P])
            qs[(k + 2) % 3].dma_start(wk_sb[:, k, 0:P], wk_r[:, k, 0:P])
            qs[k % 3].dma_start(wv_sb[:, k, :], wv_r[:, k, :])
        nc.sync.dma_start(bq_sb[:], bq2[:])
        nc.gpsimd.dma_start(bk_sb[:], bk2[:])
        nc.scalar.dma_start(bv_sb[:], bv2[:])
        nc.scalar.dma_start(v_sb[:, :, :, 64:65], vones_d[:])
        ones_sb = wp.tile([P, 64], F32R, tag="ones")
        nc.sync.dma_start(ones_sb[:], ones_d[:])
        for k in range(KO):
            qs[k % 3].dma_start(wq_sb[:, k, P:256], wq_r[:, k, P:256])
            qs[(k + 1) % 3].dma_start(wk_sb[:, k, P:256], wk_r[:, k, P:256])
        for k2 in range(MO):
            nc.gpsimd.dma_start(wo_sb[:, k2, :], wo_r[:, k2, :])

        # ---- m=0 (heads 0/1) q/k projections, k-outer: all four
        # accumulators (proj x token-half) live at once in a scoped
        # 8-bank PSUM pool so each x chunk is contracted the moment it
        # lands; the first scores issue right after the last chunk.
        with tc.tile_pool(name="ldp", bufs=4, space="PSUM") as ldp:
            ld = {}
            for pi in range(2):
                for tp2 in range(2):
                    ld[(pi, tp2)] = ldp.tile([P, HALF], F32, tag="ld",
                                             name=f"ld{pi}{tp2}")
            for k in range(KO):
                for pi in range(2):
                    w_sb = wq_sb if pi == 0 else wk_sb
                    for tp2 in range(2):
                        for j in range(2):
                            nc.tensor.matmul(
                                ld[(pi, tp2)][:, j * 512:(j + 1) * 512],
                                lhsT=w_sb[:, k, 0:P],
                                rhs=x_sb[:, k, tp2 * HALF + j * 512:
                                         tp2 * HALF + (j + 1) * 512],
                                start=(k == 0), stop=(k == KO - 1),
                            )
            with nc.allow_low_precision(reason="fp16 q/k store, fp32 accum"):
                for tp2 in range(2):
                    # k-proj evict on ACT (idle here), q-proj on DVE: the
                    # (k,tp0)+(q,tp0) pair gates the first scores
                    nc.scalar.activation(
                        kT_sb[:, 0, tp2 * HALF:(tp2 + 1) * HALF],
                        ld[(1, tp2)][:], AF.Identity, bias=bk_sb[:, 0:1])
                    nc.vector.tensor_tensor(
                        qT_sb[:, 0, tp2 * HALF:(tp2 + 1) * HALF],
                        ld[(0, tp2)][:],
                        bq_sb[:, 0:1].to_broadcast((P, HALF)),
                        ADD,
                    )

        # attention pools: score quad-buffer (4 banks), paired ctx (2),
        # dedicated background ring (2)
        sq = ctx.enter_context(tc.tile_pool(name="sq", bufs=4, space="PSUM"))
        cxp = ctx.enter_context(tc.tile_pool(name="cxp", bufs=2, space="PSUM"))
        bgp = ctx.enter_context(tc.tile_pool(name="bgp", bufs=2, space="PSUM"))

        # ---- background chunk emitters (all through bgp, deprioritized) ----

        def v_chunk(kt):
            # v projection for one key-chunk, all 4 heads ([tok, feat])
            def emit():
                ps = bgp.tile([P, 256], F32, tag="bg", name="vps")
                for k in range(KO):
                    nc.tensor.matmul(
                        ps[:],
                        lhsT=x_sb[:, k, kt * P:(kt + 1) * P],
                        rhs=wv_sb[:, k, :],
                        start=(k == 0), stop=(k == KO - 1),
                    )
                with nc.allow_low_precision(reason="fp16 v store"):
                    nc.vector.tensor_tensor(
                        v_sb[:, kt, :, 0:64], ps[:], bv_sb[:], ADD)
            return emit

        def m1qk_chunks():
            # m=1 (heads 2/3) q/k projections: one 8-matmul transient-PSUM
            # chunk per (proj, token-half, 512-block), single bias-add evict
            cl = []
            for pi in range(2):
                for tp2 in range(2):
                    for j in range(2):
                        def emit(pi=pi, tp2=tp2, j=j):
                            w_sb = wq_sb if pi == 0 else wk_sb
                            dst = qT_sb if pi == 0 else kT_sb
                            b_sb = bq_sb if pi == 0 else bk_sb
                            lo = tp2 * HALF + j * 512
                            ps = bgp.tile([P, 512], F32, tag="bg",
                                          name="m1")
                            for kk in range(KO):
                                nc.tensor.matmul(
                                    ps[:],
                                    lhsT=w_sb[:, kk, P:256],
                                    rhs=x_sb[:, kk, lo:lo + 512],
                                    start=(kk == 0),
                                    stop=(kk == KO - 1),
                                )
                            with nc.allow_low_precision(
                                    reason="fp16 q/k store"):
                                nc.vector.tensor_scalar_add(
                                    dst[:, 1, lo:lo + 512], ps[:],
                                    b_sb[:, 1:2])
                        cl.append(emit)
            return cl

        def op_chunks(qc):
            # out-projection for one query chunk (both k2 halves in PSUM),
            # gated on all 4 heads' ctx for that chunk
            cl = []
            lo = qc * QC
            for m in range(KO):
                def emit(m=m):
                    ps = bgp.tile([P, 512], F32, tag="bg", name="op")
                    for k2 in range(MO):
                        nc.tensor.matmul(
                            ps[:],
                            lhsT=wo_sb[:, k2, m * P:(m + 1) * P],
                            rhs=ctx_sb[:, k2, lo:lo + QC],
                            start=(k2 == 0), stop=(k2 == MO - 1),
                        )
                    ot = ob.tile([P, QC], F16, tag="ot", name="ot")
                    with nc.allow_low_precision(reason="fp16 partial"):
                        nc.vector.tensor_copy(ot[:], ps[:])
                    qs[m % 3].dma_start(pout_r[:, m, lo:lo + QC], ot[:])
                cl.append(emit)
            return cl

        # ---- softmax-normalize and evict one head's query chunk.
        # 1/Z (f32r) straight from the PSUM Z row, raw ctx copied to SBUF
        # (releases the ctx bank), 1/Z broadcast across partitions by a
        # K=1 ones matmul through the background ring, fused multiply.
        def normalize(qc, h, ctx_ps):
            o, prow = h // 2, 64 * (h % 2)
            rst = nm.tile([P, QC], F32R, tag="rst", name="rst")
            with nc.allow_low_precision(reason="1/Z in f32r, Z ~ O(S)"):
                nc.vector.reciprocal(rst[64:65, :], ctx_ps[64:65, :])
            traw = nm.tile([P, QC], F32, tag="traw", name="traw")
            nc.vector.tensor_copy(traw[0:64, :], ctx_ps[0:64, :])
            bc = bgp.tile([P, QC], F32, tag="bg", name="bc")
            nc.tensor.matmul(
                bc[0:64, :],
                lhsT=ones_sb[64:65, :],
                rhs=rst[64:65, :],
                start=True, stop=True,
            )
            with nc.allow_low_precision(reason="fp16 ctx store"):
                nc.vector.tensor_tensor(
                    ctx_sb[prow:prow + 64, o, qc * QC:(qc + 1) * QC],
                    traw[0:64, :],
                    bc[0:64, :],
                    MULT,
                )

        # ---- attention for one (head-pair, query-chunk); two heads
        # interleaved per key-chunk (prow 0/64 -> disjoint PE row groups,
        # overlapping on HW).  bg: background emitters drained at `rate`
        # per key-chunk (they only gate their own pool, not the scores).
        def attn_phase(pair, qc, bg, rate=1):
            ctx_t = [None, None]
            state = {"carry": 0.0}
            lo = qc * QC

            def emit_kt(kt):
                for hi in range(2):
                    h = 2 * pair + hi
                    o, prow = h // 2, 64 * (h % 2)
                    qh = qT_sb[prow:prow + 64, o, :]
                    kh = kT_sb[prow:prow + 64, o, :]
                    sp = sq.tile([P, QC], F32, tag="sp", name="sp")
                    nc.tensor.matmul(
                        sp[:],
                        lhsT=kh[:, kt * P:(kt + 1) * P],
                        rhs=qh[:, lo:lo + QC],
                        start=True, stop=True,
                    )
                    pt = pp.tile([P, QC], F16, tag="pt", name="pt")
                    nc.scalar.activation(pt[:], sp[:], AF.Exp, scale=0.125)
                    if kt == 0:
                        ctx_t[hi] = cxp.tile([P, QC], F32, tag="ctx",
                                             name="ctx")
                    nc.tensor.matmul(
                        ctx_t[hi][0:65, :],
                        lhsT=v_sb[:, kt, h, 0:65],
                        rhs=pt[:],
                        start=(kt == 0), stop=(kt == NKT - 1),
                    )
                state["carry"] += rate
                while bg and state["carry"] >= 1.0:
                    bg.pop(0)()
                    state["carry"] -= 1.0

            def finish():
                for hi in range(2):
                    normalize(qc, 2 * pair + hi, ctx_t[hi])

            return emit_kt, finish

        # ---- phase schedule: heads 0/1 sweep all four query chunks
        # first, so the m=1 projections have slack before heads 2/3 need
        # them; out-projection for chunk qc rides behind the pair-1 phase
        # that completes it.  Each phase's kt=0 is emitted before the
        # previous phase's normalize.
        vq = [v_chunk(kt) for kt in range(NKT)]
        for kt in range(VLEAD):
            vq.pop(0)()
        m1 = m1qk_chunks()
        seq = [
            (0, 0, vq, 2.0),
            (0, 1, m1, 0.5),
            (0, 2, m1, 0.5),
            (0, 3, [], 0),
            (1, 0, [], 0),
            (1, 1, op_chunks(0), 0.5),
            (1, 2, op_chunks(1), 0.5),
            (1, 3, op_chunks(2), 0.5),
        ]
        pending = None
        for pair, qcc, bg, rate in seq:
            ek, fin = attn_phase(pair, qcc, bg, rate)
            ek(0)
            if pending is not None:
                pending()           # previous phase's normalize rides after
            for kt in range(1, NKT):
                ek(kt)
            pending = fin
        pending()
        for e in op_chunks(3):                   # tail: ~6us
            e()
        if DEBUG:
            nc.sync.dma_start(dbg_q[:], qT_sb[:])
            nc.sync.dma_start(dbg_k[:], kT_sb[:])
            nc.sync.dma_start(dbg_v[:], v_sb[:])
            nc.sync.dma_start(dbg_c[:], ctx_sb[:])

    return nc


_NC_CACHE = []


def _get_nc():
    if not _NC_CACHE:
        nc = bacc.Bacc(
            "TRN2",
            target_bir_lowering=False,
            debug=False,
            enable_asserts=False,
            num_devices=NCORES,
        )
        build_mha_kernel(nc)
        nc.finalize()
        _NC_CACHE.append(nc)
    return _NC_CACHE[0]


def _shard(x, wq, bq, wk, bk, wv, bv, wo):
    in_maps = []
    f16 = np.float16
    for c in range(NCORES):
        b, hg = c // 4, c % 4
        I = slice(256 * hg, 256 * hg + 256)
        m = {
            "xT": np.ascontiguousarray(x[b].T).astype(f16),
            "wqT": np.ascontiguousarray(wq[I, :].T).astype(f16),
            "wkT": np.ascontiguousarray(wk[I, :].T).astype(f16),
            "wvT": np.ascontiguousarray(wv[I, :].T).astype(f16),
            "bq2": np.ascontiguousarray(
                bq[I].reshape(MO, P).T).astype(np.float32),
            "bk2": np.ascontiguousarray(
                bk[I].reshape(MO, P).T).astype(np.float32),
            "bv2": np.ascontiguousarray(
                np.broadcast_to(bv[I], (P, 256))).astype(np.float32),
            "woT": np.ascontiguousarray(wo[:, I].T).astype(f16),
            "vones_d": np.ones((P, NKT, HEADS, 1), f16),
            "ones_d": np.ones((P, 64), np.float32),
        }
        in_maps.append(m)
    return in_maps


def kernel(x, wq, bq, wk, bk, wv, bv, wo, bo):
    x = np.asarray(x, dtype=np.float32)
    nc = _get_nc()
    in_maps = _shard(x, np.asarray(wq), np.asarray(bq), np.asarray(wk),
                     np.asarray(bk), np.asarray(wv), np.asarray(bv),
                     np.asarray(wo))
    res = run_bass_kernel_spmd(nc, in_maps, list(range(NCORES)), trace=TRACE)
    LAST_RESULT.clear()
    LAST_RESULT["exec_time_ns"] = res.exec_time_ns
    LAST_RESULT["mean_exec_time_ns"] = getattr(res, "mean_exec_time_ns", None)

    out = np.zeros((B, S, H), dtype=np.float64)
    for c in range(NCORES):
        out[c // 4] += res.results[c]["poutT"].astype(np.float64).T
    out += np.asarray(bo, dtype=np.float64)
    return out.astype(np.float32)


# revision 26
# speedup vs baseline: 1.2001x; 1.2001x over previous
"""8-way sharded MultiHeadAttention for Trainium2 (Bass/Tile).

Problem: B=2, S=2048, H=1024, NH=16 heads of D=64.
  out = softmax((x@wq.T+bq) @ (x@wk.T+bk).T / sqrt(D)) @ (x@wv.T+bv),
  concat heads, @ wo.T + bo.

Sharding (Megatron-style tensor parallel over 8 NeuronCores):
  core c owns batch b = c//4 and the 4 heads 4*(c%4)..4*(c%4)+3
  (feature columns Ic = 256*(c%4) .. +256 of q/k/v).
  - column-parallel QKV projections, attention fully local per head,
  - row-parallel output projection producing a partial [H, S] result;
    the 4 partials per batch are summed on the host.

v3 design notes:
  - All matmul operands are fp16 (same 1 cyc/row PE rate as fp32r, half
    the DMA/SBUF footprint, 2x faster weight loads via FWL); PSUM
    accumulation stays fp32.
  - Attention runs in 512-query phases (2 head-pairs x 4 query chunks,
    16 key-chunks each).  Every attention PSUM tile is then ONE 2KB
    bank: the score buffer quadruple-buffers in 4 banks, the paired
    ctx accumulators take 2, and the remaining 2 banks form a
    DEDICATED background pool -- so the v projection, the m=1 (heads
    2/3) q/k projections, the out-projection and the 1/Z broadcasts
    never block the score/exp pipeline (in v2 they shared one ring and
    each background chunk stalled the exp stream).
  - x streams in k-chunks over 3 DGE queues; the m=0 (heads 0/1) q/k
    projections contract each chunk as it lands into 4 concurrent
    accumulators in a scoped 8-bank pool, so the first scores + exp
    issue right after the last chunk.  A dummy exp at t=0 preloads the
    ACT exp table set (~2.7us).
  - scores are computed transposed ([key, query]); the softmax sum
    folds into attn@v via a ones-augmented V ([v | 1]).  exp runs on
    the scalar engine straight out of PSUM with the 1/sqrt(D) scale
    fused.  No max-subtraction (|scores/8| < ~5.5, exp safe in fp32).
  - the head pair sits at prow 0/64, so the two K=64 score matmuls
    land in disjoint PE row groups (tile_position) and overlap on HW.
  - softmax normalization: 1/Z (DVE reciprocal, f32r) is broadcast
    across partitions by a K=1 ones matmul through the background pool
    and fused into the PSUM eviction multiply.  Each phase's first
    key-chunk is emitted before the previous phase's normalize so the
    exp stream never waits on it.
  - out-projection chunks are gated per query-chunk on the second
    head-pair's normalize and run as deprioritized background; only
    the last query chunk's 8 chunks trail the final exp (~6us tail).
"""

import sys

for _p in ("/opt/trn_rl_repo", "/root/.axon_site/_ro/trn_rl_repo"):
    if _p not in sys.path:
        sys.path.append(_p)

from contextlib import ExitStack

import numpy as np

import concourse.bass as bass
import concourse.mybir as mybir
import concourse.tile as tile
from concourse import bacc
from concourse.bass_utils import run_bass_kernel_spmd

F32 = mybir.dt.float32
F16 = mybir.dt.float16
F32R = mybir.dt.float32r
AF = mybir.ActivationFunctionType
ADD = mybir.AluOpType.add
MULT = mybir.AluOpType.mult

P = 128
B = 2
S = 2048          # tokens
H = 1024          # hidden
KO = H // P       # 8 k-chunks for the QKV projections
MO = 2            # 256 local features / 128
HEADS = 4         # heads per core
D = 64
NKT = S // P      # 16 key chunks
HALF = 1024
QC = 512          # query chunk width (one PSUM bank of fp32)
NQC = S // QC     # 4 query chunks
NCORES = 8
VPAD = 72         # per-(kt, head) v row stride in elems (65 used, 16B-aligned)
VLEAD = 2         # v-projection chunks emitted ahead of the attn@v consumer

# test.py can flip these before calling kernel()
TRACE = False
DEBUG = False
LAST_RESULT = {}


def build_mha_kernel(nc: bass.Bass):
    xT = nc.declare_dram_parameter("xT", [H, S], F16, isOutput=False)
    wqT = nc.declare_dram_parameter("wqT", [H, 256], F16, isOutput=False)
    wkT = nc.declare_dram_parameter("wkT", [H, 256], F16, isOutput=False)
    wvT = nc.declare_dram_parameter("wvT", [H, 256], F16, isOutput=False)
    bq2 = nc.declare_dram_parameter("bq2", [P, MO], F32, isOutput=False)
    bk2 = nc.declare_dram_parameter("bk2", [P, MO], F32, isOutput=False)
    bv2 = nc.declare_dram_parameter("bv2", [P, 256], F32, isOutput=False)
    woT = nc.declare_dram_parameter("woT", [256, H], F16, isOutput=False)
    vones_d = nc.declare_dram_parameter("vones_d", [P, NKT, HEADS, 1], F16,
                                        isOutput=False)
    ones_d = nc.declare_dram_parameter("ones_d", [P, 64], F32R,
                                       isOutput=False)
    poutT = nc.declare_dram_parameter("poutT", [H, S], F16, isOutput=True)
    if DEBUG:
        dbg_q = nc.declare_dram_parameter("dbg_q", [P, MO, S], F16,
                                          isOutput=True)
        dbg_k = nc.declare_dram_parameter("dbg_k", [P, MO, S], F16,
                                          isOutput=True)
        dbg_v = nc.declare_dram_parameter("dbg_v", [P, NKT, HEADS, VPAD], F16,
                                          isOutput=True)
        dbg_c = nc.declare_dram_parameter("dbg_c", [P, MO, S], F16,
                                          isOutput=True)

    xT_r = xT.rearrange("(o p) n -> p o n", p=P)        # [128, 8, 2048]
    wq_r = wqT.rearrange("(o p) m -> p o m", p=P)       # [128, 8, 256]
    wk_r = wkT.rearrange("(o p) m -> p o m", p=P)
    wv_r = wvT.rearrange("(o p) m -> p o m", p=P)
    wo_r = woT.rearrange("(o p) m -> p o m", p=P)       # [128, 2, 1024]
    pout_r = poutT.rearrange("(o p) n -> p o n", p=P)   # [128, 8, 2048]

    with tile.TileContext(nc) as tc, ExitStack() as ctx:
        xp = ctx.enter_context(tc.tile_pool(name="xp", bufs=1))
        wp = ctx.enter_context(tc.tile_pool(name="wp", bufs=1))
        qk = ctx.enter_context(tc.tile_pool(name="qk", bufs=1))
        vp = ctx.enter_context(tc.tile_pool(name="vp", bufs=1))
        cx = ctx.enter_context(tc.tile_pool(name="cx", bufs=1))
        pp = ctx.enter_context(tc.tile_pool(name="pp", bufs=8))
        nm = ctx.enter_context(tc.tile_pool(name="nm", bufs=3))
        ob = ctx.enter_context(tc.tile_pool(name="ob", bufs=6))

        x_sb = xp.tile([P, KO, S], F16)
        wq_sb = wp.tile([P, KO, 256], F16, tag="wq")
        wk_sb = wp.tile([P, KO, 256], F16, tag="wk")
        wv_sb = wp.tile([P, KO, 256], F16, tag="wv")
        wo_sb = wp.tile([P, MO, H], F16, tag="wo")
        bq_sb = wp.tile([P, MO], F32, tag="bq")
        bk_sb = wp.tile([P, MO], F32, tag="bk")
        bv_sb = wp.tile([P, 256], F32, tag="bv")

        qT_sb = qk.tile([P, MO, S], F16, tag="q")       # [feat, token]
        kT_sb = qk.tile([P, MO, S], F16, tag="k")
        v_sb = vp.tile([P, NKT, HEADS, VPAD], F16)      # [tok, kt, h, v|1|pad]
        ctx_sb = cx.tile([P, MO, S], F16)

        # ---- ACT exp table preload: one dummy exp at t=0 hides the
        # ~2.7us table-set load before the first real scores arrive
        dum = nm.tile([P, 8], F32, tag="dummy")
        nc.vector.memset(dum[0:1, :], 0.0)
        nc.scalar.activation(dum[0:1, :], dum[0:1, :], AF.Exp, scale=1.0)

        # ---- input DMA over 3 DGE queues: x chunks + the m=0 q/k weight
        # halves (they gate the first exp) + wv (gates the v trickle)
        # first; m=1 halves, wo and the ones columns behind them.
        qs = [nc.sync, nc.gpsimd, nc.scalar]
        for k in range(KO):
            qs[k % 3].dma_start(x_sb[:, k, :], xT_r[:, k, :])
            qs[(k + 1) % 3].dma_start(wq_sb[:, k, 0:P], wq_r[:, k, 0:P])
            qs[(k + 2) % 3].dma_start(wk_sb[:, k, 0:P], wk_r[:, k, 0:P])
            qs[k % 3].dma_start(wv_sb[:, k, :], wv_r[:, k, :])
        nc.sync.dma_start(bq_sb[:], bq2[:])
        nc.gpsimd.dma_start(bk_sb[:], bk2[:])
        nc.scalar.dma_start(bv_sb[:], bv2[:])
        nc.scalar.dma_start(v_sb[:, :, :, 64:65], vones_d[:])
        ones_sb = wp.tile([P, 64], F32R, tag="ones")
        nc.sync.dma_start(ones_sb[:], ones_d[:])
        for k in range(KO):
            qs[k % 3].dma_start(wq_sb[:, k, P:256], wq_r[:, k, P:256])
            qs[(k + 1) % 3].dma_start(wk_sb[:, k, P:256], wk_r[:, k, P:256])
        for k2 in range(MO):
            nc.gpsimd.dma_start(wo_sb[:, k2, :], wo_r[:, k2, :])

        # ---- m=0 (heads 0/1) q/k projections, k-outer: all four
        # accumulators (proj x token-half) live at once in a scoped
        # 8-bank PSUM pool so each x chunk is contracted the moment it
        # lands; the first scores issue right after the last chunk.
        with tc.tile_pool(name="ldp", bufs=4, space="PSUM") as ldp:
            ld = {}
            for pi in range(2):
                for tp2 in range(2):
                    ld[(pi, tp2)] = ldp.tile([P, HALF], F32, tag="ld",
                                             name=f"ld{pi}{tp2}")
            for k in range(KO):
                for pi in range(2):
                    w_sb = wq_sb if pi == 0 else wk_sb
                    for tp2 in range(2):
                        for j in range(2):
                            nc.tensor.matmul(
                                ld[(pi, tp2)][:, j * 512:(j + 1) * 512],
                                lhsT=w_sb[:, k, 0:P],
                                rhs=x_sb[:, k, tp2 * HALF + j * 512:
                                         tp2 * HALF + (j + 1) * 512],
                                start=(k == 0), stop=(k == KO - 1),
                            )
            with nc.allow_low_precision(reason="fp16 q/k store, fp32 accum"):
                for tp2 in range(2):
                    # k-proj evict on ACT (idle here), q-proj on DVE: the
                    # (k,tp0)+(q,tp0) pair gates the first scores
                    nc.scalar.activation(
                        kT_sb[:, 0, tp2 * HALF:(tp2 + 1) * HALF],
                        ld[(1, tp2)][:], AF.Identity, bias=bk_sb[:, 0:1])
                    nc.vector.tensor_tensor(
                        qT_sb[:, 0, tp2 * HALF:(tp2 + 1) * HALF],
                        ld[(0, tp2)][:],
                        bq_sb[:, 0:1].to_broadcast((P, HALF)),
                        ADD,
                    )

        # attention pools: score quad-buffer (4 banks), paired ctx (2),
        # dedicated background ring (2)
        sq = ctx.enter_context(tc.tile_pool(name="sq", bufs=2, space="PSUM"))
        cxp = ctx.enter_context(tc.tile_pool(name="cxp", bufs=2, space="PSUM"))
        bgp = ctx.enter_context(tc.tile_pool(name="bgp", bufs=2, space="PSUM"))

        # ---- background chunk emitters (all through bgp, deprioritized) ----

        def v_chunk(kt, pr):
            # v projection for one key-chunk, one head-pair ([tok, feat]);
            # the pair split keeps heads 2/3's v out of the PE-saturated
            # first phase
            def emit():
                ps = bgp.tile([P, 256], F32, tag="bg", name="vps")
                for k in range(KO):
                    nc.tensor.matmul(
                        ps[:, 0:P],
                        lhsT=x_sb[:, k, kt * P:(kt + 1) * P],
                        rhs=wv_sb[:, k, pr * P:(pr + 1) * P],
                        start=(k == 0), stop=(k == KO - 1),
                    )
                with nc.allow_low_precision(reason="fp16 v store"):
                    nc.vector.tensor_tensor(
                        v_sb[:, kt, 2 * pr:2 * pr + 2, 0:64],
                        ps[:, 0:P],
                        bv_sb[:, pr * P:(pr + 1) * P],
                        ADD)
            return emit

        def m1qk_chunks():
            # m=1 (heads 2/3) q/k projections: one 8-matmul transient-PSUM
            # chunk per (proj, token-half, 512-block), single bias-add evict
            cl = []
            for pi in range(2):
                for tp2 in range(2):
                    for j in range(2):
                        def emit(pi=pi, tp2=tp2, j=j):
                            w_sb = wq_sb if pi == 0 else wk_sb
                            dst = qT_sb if pi == 0 else kT_sb
                            b_sb = bq_sb if pi == 0 else bk_sb
                            lo = tp2 * HALF + j * 512
                            ps = bgp.tile([P, 512], F32, tag="bg",
                                          name="m1")
                            for kk in range(KO):
                                nc.tensor.matmul(
                                    ps[:],
                                    lhsT=w_sb[:, kk, P:256],
                                    rhs=x_sb[:, kk, lo:lo + 512],
                                    start=(kk == 0),
                                    stop=(kk == KO - 1),
                                )
                            with nc.allow_low_precision(
                                    reason="fp16 q/k store"):
                                nc.vector.tensor_scalar_add(
                                    dst[:, 1, lo:lo + 512], ps[:],
                                    b_sb[:, 1:2])
                        cl.append(emit)
            return cl

        def op_chunks(qc, use_act=False):
            # out-projection for one query chunk (both k2 halves in PSUM),
            # gated on all 4 heads' ctx for that chunk.  use_act alternates
            # evictions between DVE and the (idle in the tail) ACT engine.
            cl = []
            lo = qc * QC
            for m in range(KO):
                def emit(m=m):
                    ps = bgp.tile([P, 512], F32, tag="bg", name="op")
                    for k2 in range(MO):
                        nc.tensor.matmul(
                            ps[:],
                            lhsT=wo_sb[:, k2, m * P:(m + 1) * P],
                            rhs=ctx_sb[:, k2, lo:lo + QC],
                            start=(k2 == 0), stop=(k2 == MO - 1),
                        )
                    ot = ob.tile([P, QC], F16, tag="ot", name="ot")
                    if use_act and m % 2 == 1:
                        nc.scalar.copy(ot[:], ps[:])
                    else:
                        with nc.allow_low_precision(reason="fp16 partial"):
                            nc.vector.tensor_copy(ot[:], ps[:])
                    qs[m % 3].dma_start(pout_r[:, m, lo:lo + QC], ot[:])
                cl.append(emit)
            return cl

        # ---- softmax-normalize and evict one head's query chunk.
        # 1/Z (f32r) straight from the PSUM Z row, raw ctx copied to SBUF
        # (releases the ctx bank), 1/Z broadcast across partitions by a
        # K=1 ones matmul through the background ring, fused multiply.
        def normalize(qc, h, ctx_ps):
            o, prow = h // 2, 64 * (h % 2)
            rst = nm.tile([P, QC], F32R, tag="rst", name="rst")
            with nc.allow_low_precision(reason="1/Z in f32r, Z ~ O(S)"):
                nc.vector.reciprocal(rst[64:65, :], ctx_ps[64:65, :])
            traw = nm.tile([P, QC], F32, tag="traw", name="traw")
            nc.vector.tensor_copy(traw[0:64, :], ctx_ps[0:64, :])
            bc = bgp.tile([P, QC], F32, tag="bg", name="bc")
            nc.tensor.matmul(
                bc[0:64, :],
                lhsT=ones_sb[64:65, :],
                rhs=rst[64:65, :],
                start=True, stop=True,
            )
            with nc.allow_low_precision(reason="fp16 ctx store"):
                nc.vector.tensor_tensor(
                    ctx_sb[prow:prow + 64, o, qc * QC:(qc + 1) * QC],
                    traw[0:64, :],
                    bc[0:64, :],
                    MULT,
                )

        # ---- attention for one (head-pair, query-chunk); two heads
        # interleaved per key-chunk (prow 0/64 -> disjoint PE row groups,
        # overlapping on HW).  bg: background emitters drained at `rate`
        # per key-chunk (they only gate their own pool, not the scores).
        def attn_phase(pair, qc, bg, rate=1):
            # super-steps of TWO key-chunks: the score tile is a 2-bank
            # [P, 2, QC] super-tile so each exp stays 1024 elements wide
            # (the real ACT per-instruction overhead is ~352 cycles)
            ctx_t = [None, None]
            state = {"carry": 0.0}
            lo = qc * QC

            def emit_kt(kt2):
                kt0 = 2 * kt2
                for hi in range(2):
                    h = 2 * pair + hi
                    o, prow = h // 2, 64 * (h % 2)
                    qh = qT_sb[prow:prow + 64, o, :]
                    kh = kT_sb[prow:prow + 64, o, :]
                    sp = sq.tile([P, 2, QC], F32, tag="sp", name="sp")
                    for ktl in range(2):
                        kt = kt0 + ktl
                        nc.tensor.matmul(
                            sp[:, ktl, :],
                            lhsT=kh[:, kt * P:(kt + 1) * P],
                            rhs=qh[:, lo:lo + QC],
                            start=True, stop=True,
                        )
                    pt = pp.tile([P, 2, QC], F16, tag="pt", name="pt")
                    nc.scalar.activation(pt[:], sp[:], AF.Exp, scale=0.125)
                    if kt2 == 0:
                        ctx_t[hi] = cxp.tile([P, QC], F32, tag="ctx",
                                             name="ctx")
                    for ktl in range(2):
                        kt = kt0 + ktl
                        nc.tensor.matmul(
                            ctx_t[hi][0:65, :],
                            lhsT=v_sb[:, kt, h, 0:65],
                            rhs=pt[:, ktl, :],
                            start=(kt == 0), stop=(kt == NKT - 1),
                        )
                state["carry"] += rate
                while bg and state["carry"] >= 1.0:
                    bg.pop(0)()
                    state["carry"] -= 1.0

            def finish():
                for hi in range(2):
                    normalize(qc, 2 * pair + hi, ctx_t[hi])

            return emit_kt, finish

        # ---- phase schedule: heads 0/1 sweep all four query chunks
        # first, so the m=1 projections have slack before heads 2/3 need
        # them; out-projection for chunk qc rides behind the pair-1 phase
        # that completes it.  Each phase's kt=0 is emitted before the
        # previous phase's normalize.
        vq0 = [v_chunk(kt, 0) for kt in range(NKT)]
        for kt in range(VLEAD):
            vq0.pop(0)()
        vq1 = [v_chunk(kt, 1) for kt in range(NKT)]
        m1 = m1qk_chunks()
        seq = [
            (0, 0, vq0, 4.0),
            (0, 1, vq1, 2.0),
            (0, 2, m1, 1.0),
            (0, 3, [], 0),
            (1, 0, [], 0),
            (1, 1, op_chunks(0), 1.0),
            (1, 2, op_chunks(1), 1.0),
            (1, 3, op_chunks(2), 1.0),
        ]
        pending = None
        for pair, qcc, bg, rate in seq:
            ek, fin = attn_phase(pair, qcc, bg, rate)
            ek(0)
            if pending is not None:
                pending()           # previous phase's normalize rides after
            for kt2 in range(1, NKT // 2):
                ek(kt2)
            pending = fin
        pending()
        for e in op_chunks(3, use_act=True):     # tail
            e()
        if DEBUG:
            nc.sync.dma_start(dbg_q[:], qT_sb[:])
            nc.sync.dma_start(dbg_k[:], kT_sb[:])
            nc.sync.dma_start(dbg_v[:], v_sb[:])
            nc.sync.dma_start(dbg_c[:], ctx_sb[:])

    return nc


_NC_CACHE = []


def _get_nc():
    if not _NC_CACHE:
        nc = bacc.Bacc(
            "TRN2",
            target_bir_lowering=False,
            debug=False,
            enable_asserts=False,
            num_devices=NCORES,
        )
        build_mha_kernel(nc)
        nc.finalize()
        _NC_CACHE.append(nc)
    return _NC_CACHE[0]


def _shard(x, wq, bq, wk, bk, wv, bv, wo):
    in_maps = []
    f16 = np.float16
    for c in range(NCORES):
        b, hg = c // 4, c % 4
        I = slice(256 * hg, 256 * hg + 256)
        m = {
            "xT": np.ascontiguousarray(x[b].T).astype(f16),
            "wqT": np.ascontiguousarray(wq[I, :].T).astype(f16),
            "wkT": np.ascontiguousarray(wk[I, :].T).astype(f16),
            "wvT": np.ascontiguousarray(wv[I, :].T).astype(f16),
            "bq2": np.ascontiguousarray(
                bq[I].reshape(MO, P).T).astype(np.float32),
            "bk2": np.ascontiguousarray(
                bk[I].reshape(MO, P).T).astype(np.float32),
            "bv2": np.ascontiguousarray(
                np.broadcast_to(bv[I], (P, 256))).astype(np.float32),
            "woT": np.ascontiguousarray(wo[:, I].T).astype(f16),
            "vones_d": np.ones((P, NKT, HEADS, 1), f16),
            "ones_d": np.ones((P, 64), np.float32),
        }
        in_maps.append(m)
    return in_maps


def kernel(x, wq, bq, wk, bk, wv, bv, wo, bo):
    x = np.asarray(x, dtype=np.float32)
    nc = _get_nc()
    in_maps = _shard(x, np.asarray(wq), np.asarray(bq), np.asarray(wk),
                     np.asarray(bk), np.asarray(wv), np.asarray(bv),
                     np.asarray(wo))
    res = run_bass_kernel_spmd(nc, in_maps, list(range(NCORES)), trace=TRACE)
    LAST_RESULT.clear()
    LAST_RESULT["exec_time_ns"] = res.exec_time_ns
    LAST_RESULT["mean_exec_time_ns"] = getattr(res, "mean_exec_time_ns", None)

    out = np.zeros((B, S, H), dtype=np.float64)
    for c in range(NCORES):
        out[c // 4] += res.results[c]["poutT"].astype(np.float64).T
    out += np.asarray(bo, dtype=np.float64)
    return out.astype(np.float32)


# revision 28
# speedup vs baseline: 1.3046x; 1.0871x over previous
"""8-way sharded MultiHeadAttention for Trainium2 (Bass/Tile).

Problem: B=2, S=2048, H=1024, NH=16 heads of D=64.
  out = softmax((x@wq.T+bq) @ (x@wk.T+bk).T / sqrt(D)) @ (x@wv.T+bv),
  concat heads, @ wo.T + bo.

Sharding (Megatron-style tensor parallel over 8 NeuronCores):
  core c owns batch b = c//4 and the 4 heads 4*(c%4)..4*(c%4)+3
  (feature columns Ic = 256*(c%4) .. +256 of q/k/v).
  - column-parallel QKV projections, attention fully local per head,
  - row-parallel output projection producing a partial [H, S] result;
    the 4 partials per batch are summed on the host.

v3 design notes:
  - All matmul operands are fp16 (same 1 cyc/row PE rate as fp32r, half
    the DMA/SBUF footprint, 2x faster weight loads via FWL); PSUM
    accumulation stays fp32.
  - Attention runs in 512-query phases (2 head-pairs x 4 query chunks,
    16 key-chunks each).  Every attention PSUM tile is then ONE 2KB
    bank: the score buffer quadruple-buffers in 4 banks, the paired
    ctx accumulators take 2, and the remaining 2 banks form a
    DEDICATED background pool -- so the v projection, the m=1 (heads
    2/3) q/k projections, the out-projection and the 1/Z broadcasts
    never block the score/exp pipeline (in v2 they shared one ring and
    each background chunk stalled the exp stream).
  - x streams in k-chunks over 3 DGE queues; the m=0 (heads 0/1) q/k
    projections contract each chunk as it lands into 4 concurrent
    accumulators in a scoped 8-bank pool, so the first scores + exp
    issue right after the last chunk.  A dummy exp at t=0 preloads the
    ACT exp table set (~2.7us).
  - scores are computed transposed ([key, query]); the softmax sum
    folds into attn@v via a ones-augmented V ([v | 1]).  exp runs on
    the scalar engine straight out of PSUM with the 1/sqrt(D) scale
    fused.  No max-subtraction (|scores/8| < ~5.5, exp safe in fp32).
  - the head pair sits at prow 0/64, so the two K=64 score matmuls
    land in disjoint PE row groups (tile_position) and overlap on HW.
  - softmax normalization: 1/Z (DVE reciprocal, f32r) is broadcast
    across partitions by a K=1 ones matmul through the background pool
    and fused into the PSUM eviction multiply.  Each phase's first
    key-chunk is emitted before the previous phase's normalize so the
    exp stream never waits on it.
  - out-projection chunks are gated per query-chunk on the second
    head-pair's normalize and run as deprioritized background; only
    the last query chunk's 8 chunks trail the final exp (~6us tail).
"""

import sys

for _p in ("/opt/trn_rl_repo", "/root/.axon_site/_ro/trn_rl_repo"):
    if _p not in sys.path:
        sys.path.append(_p)

from contextlib import ExitStack

import numpy as np

import concourse.bass as bass
import concourse.mybir as mybir
import concourse.tile as tile
from concourse import bacc
from concourse.bass_utils import run_bass_kernel_spmd

F32 = mybir.dt.float32
F16 = mybir.dt.float16
F32R = mybir.dt.float32r
AF = mybir.ActivationFunctionType
ADD = mybir.AluOpType.add
MULT = mybir.AluOpType.mult

P = 128
B = 2
S = 2048          # tokens
H = 1024          # hidden
KO = H // P       # 8 k-chunks for the QKV projections
MO = 2            # 256 local features / 128
HEADS = 4         # heads per core
D = 64
NKT = S // P      # 16 key chunks
HALF = 1024
QC = 512          # query chunk width (one PSUM bank of fp32)
NQC = S // QC     # 4 query chunks
NCORES = 8
VPAD = 72         # per-(kt, head) v row stride in elems (65 used, 16B-aligned)
VLEAD = 4         # v-projection chunks emitted ahead of the attn@v consumer

# test.py can flip these before calling kernel()
TRACE = False
DEBUG = False
LAST_RESULT = {}


def build_mha_kernel(nc: bass.Bass):
    xT = nc.declare_dram_parameter("xT", [H, S], F16, isOutput=False)
    wqT = nc.declare_dram_parameter("wqT", [H, 256], F16, isOutput=False)
    wkT = nc.declare_dram_parameter("wkT", [H, 256], F16, isOutput=False)
    wvT = nc.declare_dram_parameter("wvT", [H, 256], F16, isOutput=False)
    bq2 = nc.declare_dram_parameter("bq2", [P, MO], F32, isOutput=False)
    bk2 = nc.declare_dram_parameter("bk2", [P, MO], F32, isOutput=False)
    bv2 = nc.declare_dram_parameter("bv2", [P, 256], F32, isOutput=False)
    woT = nc.declare_dram_parameter("woT", [256, H], F16, isOutput=False)
    vones_d = nc.declare_dram_parameter("vones_d", [P, NKT, HEADS, 1], F16,
                                        isOutput=False)
    ones_d = nc.declare_dram_parameter("ones_d", [P, 64], F32R,
                                       isOutput=False)
    poutT = nc.declare_dram_parameter("poutT", [H, S], F16, isOutput=True)
    if DEBUG:
        dbg_q = nc.declare_dram_parameter("dbg_q", [P, MO, S], F16,
                                          isOutput=True)
        dbg_k = nc.declare_dram_parameter("dbg_k", [P, MO, S], F16,
                                          isOutput=True)
        dbg_v = nc.declare_dram_parameter("dbg_v", [P, NKT, HEADS, VPAD], F16,
                                          isOutput=True)
        dbg_c = nc.declare_dram_parameter("dbg_c", [P, MO, S], F16,
                                          isOutput=True)

    xT_r = xT.rearrange("(o p) n -> p o n", p=P)        # [128, 8, 2048]
    wq_r = wqT.rearrange("(o p) m -> p o m", p=P)       # [128, 8, 256]
    wk_r = wkT.rearrange("(o p) m -> p o m", p=P)
    wv_r = wvT.rearrange("(o p) m -> p o m", p=P)
    wo_r = woT.rearrange("(o p) m -> p o m", p=P)       # [128, 2, 1024]
    pout_r = poutT.rearrange("(o p) n -> p o n", p=P)   # [128, 8, 2048]

    with tile.TileContext(nc) as tc, ExitStack() as ctx:
        xp = ctx.enter_context(tc.tile_pool(name="xp", bufs=1))
        wp = ctx.enter_context(tc.tile_pool(name="wp", bufs=1))
        qk = ctx.enter_context(tc.tile_pool(name="qk", bufs=1))
        vp = ctx.enter_context(tc.tile_pool(name="vp", bufs=1))
        cx = ctx.enter_context(tc.tile_pool(name="cx", bufs=1))
        pp = ctx.enter_context(tc.tile_pool(name="pp", bufs=8))
        nm = ctx.enter_context(tc.tile_pool(name="nm", bufs=3))
        ob = ctx.enter_context(tc.tile_pool(name="ob", bufs=6))

        x_sb = xp.tile([P, KO, S], F16)
        wq_sb = wp.tile([P, KO, 256], F16, tag="wq")
        wk_sb = wp.tile([P, KO, 256], F16, tag="wk")
        wv_sb = wp.tile([P, KO, 256], F16, tag="wv")
        wo_sb = wp.tile([P, MO, H], F16, tag="wo")
        bq_sb = wp.tile([P, MO], F32, tag="bq")
        bk_sb = wp.tile([P, MO], F32, tag="bk")
        bv_sb = wp.tile([P, 256], F32, tag="bv")

        qT_sb = qk.tile([P, MO, S], F16, tag="q")       # [feat, token]
        kT_sb = qk.tile([P, MO, S], F16, tag="k")
        v_sb = vp.tile([P, NKT, HEADS, VPAD], F16)      # [tok, kt, h, v|1|pad]
        ctx_sb = cx.tile([P, MO, S], F16)

        # ---- ACT exp table preload: one dummy exp at t=0 hides the
        # ~2.7us table-set load before the first real scores arrive
        dum = nm.tile([P, 8], F32, tag="dummy")
        nc.vector.memset(dum[0:1, :], 0.0)
        nc.scalar.activation(dum[0:1, :], dum[0:1, :], AF.Exp, scale=1.0)

        # ---- input DMA over 3 DGE queues: x chunks + the m=0 q/k weight
        # halves (they gate the first exp) + wv (gates the v trickle)
        # first; m=1 halves, wo and the ones columns behind them.
        qs = [nc.sync, nc.gpsimd, nc.scalar]
        for k in range(KO):
            qs[k % 3].dma_start(x_sb[:, k, :], xT_r[:, k, :])
            qs[(k + 1) % 3].dma_start(wq_sb[:, k, 0:P], wq_r[:, k, 0:P])
            qs[(k + 2) % 3].dma_start(wk_sb[:, k, 0:P], wk_r[:, k, 0:P])
            qs[k % 3].dma_start(wv_sb[:, k, :], wv_r[:, k, :])
        nc.sync.dma_start(bq_sb[:], bq2[:])
        nc.gpsimd.dma_start(bk_sb[:], bk2[:])
        nc.scalar.dma_start(bv_sb[:], bv2[:])
        nc.scalar.dma_start(v_sb[:, :, :, 64:65], vones_d[:])
        ones_sb = wp.tile([P, 64], F32R, tag="ones")
        nc.sync.dma_start(ones_sb[:], ones_d[:])
        for k in range(KO):
            qs[k % 3].dma_start(wq_sb[:, k, P:256], wq_r[:, k, P:256])
            qs[(k + 1) % 3].dma_start(wk_sb[:, k, P:256], wk_r[:, k, P:256])
        for k2 in range(MO):
            nc.gpsimd.dma_start(wo_sb[:, k2, :], wo_r[:, k2, :])

        # ---- m=0 (heads 0/1) q/k projections, k-outer: all four
        # accumulators (proj x token-half) live at once in a scoped
        # 8-bank PSUM pool so each x chunk is contracted the moment it
        # lands; the first scores issue right after the last chunk.
        with tc.tile_pool(name="ldp", bufs=4, space="PSUM") as ldp:
            ld = {}
            for pi in range(2):
                for tp2 in range(2):
                    ld[(pi, tp2)] = ldp.tile([P, HALF], F32, tag="ld",
                                             name=f"ld{pi}{tp2}")
            for k in range(KO):
                for pi in range(2):
                    w_sb = wq_sb if pi == 0 else wk_sb
                    for tp2 in range(2):
                        for j in range(2):
                            nc.tensor.matmul(
                                ld[(pi, tp2)][:, j * 512:(j + 1) * 512],
                                lhsT=w_sb[:, k, 0:P],
                                rhs=x_sb[:, k, tp2 * HALF + j * 512:
                                         tp2 * HALF + (j + 1) * 512],
                                start=(k == 0), stop=(k == KO - 1),
                            )
            with nc.allow_low_precision(reason="fp16 q/k store, fp32 accum"):
                for tp2 in range(2):
                    # k-proj evict on ACT (idle here), q-proj on DVE: the
                    # (k,tp0)+(q,tp0) pair gates the first scores
                    nc.scalar.activation(
                        kT_sb[:, 0, tp2 * HALF:(tp2 + 1) * HALF],
                        ld[(1, tp2)][:], AF.Identity, bias=bk_sb[:, 0:1])
                    nc.vector.tensor_tensor(
                        qT_sb[:, 0, tp2 * HALF:(tp2 + 1) * HALF],
                        ld[(0, tp2)][:],
                        bq_sb[:, 0:1].to_broadcast((P, HALF)),
                        ADD,
                    )

        # attention pools: score quad-buffer (4 banks), paired ctx (2),
        # dedicated background ring (2)
        sq = ctx.enter_context(tc.tile_pool(name="sq", bufs=2, space="PSUM"))
        cxp = ctx.enter_context(tc.tile_pool(name="cxp", bufs=2, space="PSUM"))
        bgp = ctx.enter_context(tc.tile_pool(name="bgp", bufs=2, space="PSUM"))

        # ---- background chunk emitters (all through bgp, deprioritized) ----

        def v_chunk(kt, pr):
            # v projection for one key-chunk, one head-pair ([tok, feat]);
            # the pair split keeps heads 2/3's v out of the PE-saturated
            # first phase
            def emit():
                ps = bgp.tile([P, 256], F32, tag="bg", name="vps")
                for k in range(KO):
                    nc.tensor.matmul(
                        ps[:, 0:P],
                        lhsT=x_sb[:, k, kt * P:(kt + 1) * P],
                        rhs=wv_sb[:, k, pr * P:(pr + 1) * P],
                        start=(k == 0), stop=(k == KO - 1),
                    )
                with nc.allow_low_precision(reason="fp16 v store"):
                    nc.vector.tensor_tensor(
                        v_sb[:, kt, 2 * pr:2 * pr + 2, 0:64],
                        ps[:, 0:P],
                        bv_sb[:, pr * P:(pr + 1) * P],
                        ADD)
            return emit

        def m1qk_chunks():
            # m=1 (heads 2/3) q/k projections: one 8-matmul transient-PSUM
            # chunk per (proj, token-half, 512-block), single bias-add evict
            cl = []
            for pi in range(2):
                for tp2 in range(2):
                    for j in range(2):
                        def emit(pi=pi, tp2=tp2, j=j):
                            w_sb = wq_sb if pi == 0 else wk_sb
                            dst = qT_sb if pi == 0 else kT_sb
                            b_sb = bq_sb if pi == 0 else bk_sb
                            lo = tp2 * HALF + j * 512
                            ps = bgp.tile([P, 512], F32, tag="bg",
                                          name="m1")
                            for kk in range(KO):
                                nc.tensor.matmul(
                                    ps[:],
                                    lhsT=w_sb[:, kk, P:256],
                                    rhs=x_sb[:, kk, lo:lo + 512],
                                    start=(kk == 0),
                                    stop=(kk == KO - 1),
                                )
                            with nc.allow_low_precision(
                                    reason="fp16 q/k store"):
                                nc.vector.tensor_scalar_add(
                                    dst[:, 1, lo:lo + 512], ps[:],
                                    b_sb[:, 1:2])
                        cl.append(emit)
            return cl

        def op_chunks(qc, use_act=False):
            # out-projection for one query chunk (both k2 halves in PSUM),
            # gated on all 4 heads' ctx for that chunk.  use_act alternates
            # evictions between DVE and the (idle in the tail) ACT engine.
            cl = []
            lo = qc * QC
            for m in range(KO):
                def emit(m=m):
                    ps = bgp.tile([P, 512], F32, tag="bg", name="op")
                    for k2 in range(MO):
                        nc.tensor.matmul(
                            ps[:],
                            lhsT=wo_sb[:, k2, m * P:(m + 1) * P],
                            rhs=ctx_sb[:, k2, lo:lo + QC],
                            start=(k2 == 0), stop=(k2 == MO - 1),
                        )
                    ot = ob.tile([P, QC], F16, tag="ot", name="ot")
                    if use_act and m % 2 == 1:
                        nc.scalar.copy(ot[:], ps[:])
                    else:
                        with nc.allow_low_precision(reason="fp16 partial"):
                            nc.vector.tensor_copy(ot[:], ps[:])
                    qs[m % 3].dma_start(pout_r[:, m, lo:lo + QC], ot[:])
                cl.append(emit)
            return cl

        # ---- softmax-normalize and evict one head's query chunk.
        # 1/Z (f32r) straight from the PSUM Z row, raw ctx copied to SBUF
        # (releases the ctx bank), 1/Z broadcast across partitions by a
        # K=1 ones matmul through the background ring, fused multiply.
        def normalize(qc, h, ctx_ps):
            o, prow = h // 2, 64 * (h % 2)
            rst = nm.tile([P, QC], F32R, tag="rst", name="rst")
            with nc.allow_low_precision(reason="1/Z in f32r, Z ~ O(S)"):
                nc.vector.reciprocal(rst[64:65, :], ctx_ps[64:65, :])
            traw = nm.tile([P, QC], F32, tag="traw", name="traw")
            nc.vector.tensor_copy(traw[0:64, :], ctx_ps[0:64, :])
            bc = bgp.tile([P, QC], F32, tag="bg", name="bc")
            nc.tensor.matmul(
                bc[0:64, :],
                lhsT=ones_sb[64:65, :],
                rhs=rst[64:65, :],
                start=True, stop=True,
            )
            with nc.allow_low_precision(reason="fp16 ctx store"):
                nc.vector.tensor_tensor(
                    ctx_sb[prow:prow + 64, o, qc * QC:(qc + 1) * QC],
                    traw[0:64, :],
                    bc[0:64, :],
                    MULT,
                )

        # ---- attention for one (head-pair, query-chunk); two heads
        # interleaved per key-chunk (prow 0/64 -> disjoint PE row groups,
        # overlapping on HW).  bg: background emitters drained at `rate`
        # per key-chunk (they only gate their own pool, not the scores).
        def attn_phase(pair, qc, bg, rate=1):
            # super-steps of TWO key-chunks: the score tile is a 2-bank
            # [P, 2, QC] super-tile so each exp stays 1024 elements wide
            # (the real ACT per-instruction overhead is ~352 cycles)
            ctx_t = [None, None]
            state = {"carry": 0.0}
            lo = qc * QC

            def emit_kt(kt2):
                kt0 = 2 * kt2
                for hi in range(2):
                    h = 2 * pair + hi
                    o, prow = h // 2, 64 * (h % 2)
                    qh = qT_sb[prow:prow + 64, o, :]
                    kh = kT_sb[prow:prow + 64, o, :]
                    sp = sq.tile([P, 2, QC], F32, tag="sp", name="sp")
                    for ktl in range(2):
                        kt = kt0 + ktl
                        nc.tensor.matmul(
                            sp[:, ktl, :],
                            lhsT=kh[:, kt * P:(kt + 1) * P],
                            rhs=qh[:, lo:lo + QC],
                            start=True, stop=True,
                        )
                    pt = pp.tile([P, 2, QC], F16, tag="pt", name="pt")
                    nc.scalar.activation(pt[:], sp[:], AF.Exp, scale=0.125)
                    if kt2 == 0:
                        ctx_t[hi] = cxp.tile([P, QC], F32, tag="ctx",
                                             name="ctx")
                    for ktl in range(2):
                        kt = kt0 + ktl
                        nc.tensor.matmul(
                            ctx_t[hi][0:65, :],
                            lhsT=v_sb[:, kt, h, 0:65],
                            rhs=pt[:, ktl, :],
                            start=(kt == 0), stop=(kt == NKT - 1),
                        )
                state["carry"] += rate
                while bg and state["carry"] >= 1.0:
                    bg.pop(0)()
                    state["carry"] -= 1.0

            def finish():
                for hi in range(2):
                    normalize(qc, 2 * pair + hi, ctx_t[hi])

            return emit_kt, finish

        # ---- phase schedule: heads 0/1 sweep all four query chunks
        # first, so the m=1 projections have slack before heads 2/3 need
        # them; out-projection for chunk qc rides behind the pair-1 phase
        # that completes it.  Each phase's kt=0 is emitted before the
        # previous phase's normalize.
        vq0 = [v_chunk(kt, 0) for kt in range(NKT)]
        for kt in range(VLEAD):
            vq0.pop(0)()
        vq1 = [v_chunk(kt, 1) for kt in range(NKT)]
        m1 = m1qk_chunks()
        seq = [
            (0, 0, vq0, 3.0),
            (0, 1, vq1, 3.0),
            (0, 2, m1, 1.0),
            (0, 3, [], 0),
            (1, 0, [], 0),
            (1, 1, op_chunks(0), 1.0),
            (1, 2, op_chunks(1), 1.0),
            (1, 3, op_chunks(2), 1.0),
        ]
        pending = None
        for pair, qcc, bg, rate in seq:
            ek, fin = attn_phase(pair, qcc, bg, rate)
            ek(0)
            if pending is not None:
                pending()           # previous phase's normalize rides after
            for kt2 in range(1, NKT // 2):
                ek(kt2)
            pending = fin
        pending()
        for e in op_chunks(3, use_act=True):     # tail
            e()
        if DEBUG:
            nc.sync.dma_start(dbg_q[:], qT_sb[:])
            nc.sync.dma_start(dbg_k[:], kT_sb[:])
            nc.sync.dma_start(dbg_v[:], v_sb[:])
            nc.sync.dma_start(dbg_c[:], ctx_sb[:])

    return nc


_NC_CACHE = []


def _get_nc():
    if not _NC_CACHE:
        nc = bacc.Bacc(
            "TRN2",
            target_bir_lowering=False,
            debug=False,
            enable_asserts=False,
            num_devices=NCORES,
        )
        build_mha_kernel(nc)
        nc.finalize()
        _NC_CACHE.append(nc)
    return _NC_CACHE[0]


def _shard(x, wq, bq, wk, bk, wv, bv, wo):
    in_maps = []
    f16 = np.float16
    for c in range(NCORES):
        b, hg = c // 4, c % 4
        I = slice(256 * hg, 256 * hg + 256)
        m = {
            "xT": np.ascontiguousarray(x[b].T).astype(f16),
            "wqT": np.ascontiguousarray(wq[I, :].T).astype(f16),
            "wkT": np.ascontiguousarray(wk[I, :].T).astype(f16),
            "wvT": np.ascontiguousarray(wv[I, :].T).astype(f16),
            "bq2": np.ascontiguousarray(
                bq[I].reshape(MO, P).T).astype(np.float32),
            "bk2": np.ascontiguousarray(
                bk[I].reshape(MO, P).T).astype(np.float32),
            "bv2": np.ascontiguousarray(
                np.broadcast_to(bv[I], (P, 256))).astype(np.float32),
            "woT": np.ascontiguousarray(wo[:, I].T).astype(f16),
            "vones_d": np.ones((P, NKT, HEADS, 1), f16),
            "ones_d": np.ones((P, 64), np.float32),
        }
        in_maps.append(m)
    return in_maps


def kernel(x, wq, bq, wk, bk, wv, bv, wo, bo):
    x = np.asarray(x, dtype=np.float32)
    nc = _get_nc()
    in_maps = _shard(x, np.asarray(wq), np.asarray(bq), np.asarray(wk),
                     np.asarray(bk), np.asarray(wv), np.asarray(bv),
                     np.asarray(wo))
    res = run_bass_kernel_spmd(nc, in_maps, list(range(NCORES)), trace=TRACE)
    LAST_RESULT.clear()
    LAST_RESULT["exec_time_ns"] = res.exec_time_ns
    LAST_RESULT["mean_exec_time_ns"] = getattr(res, "mean_exec_time_ns", None)

    out = np.zeros((B, S, H), dtype=np.float64)
    for c in range(NCORES):
        out[c // 4] += res.results[c]["poutT"].astype(np.float64).T
    out += np.asarray(bo, dtype=np.float64)
    return out.astype(np.float32)
